# revision 16
# baseline (speedup 1.0000x reference)
"""Trainium2 Bass kernel for batched DotProductAttention with
(out = sum_q(softmax(QK^T/sqrt(d)) @ V), aggr_max = argmax_q softmax).

Shards batch dim B=16 across 8 NeuronCores (2 batches/core), pure data
parallel. See bottom for the host-side `kernel(**inputs)` entry point.

Per-core algorithm (natural layout, scores s[q,k] with q on partitions):
  pass A (per q-tile t of 128 q):
    PE:  s = Q_t K^T  (accumulated over 4 d-chunks of 128) -> PSUM
    ACT: e_t = exp(s/sqrt(d)) with fused accum_out -> row sums r_t
    DVE: u_t = 1/r_t ;  amax = max(amax, e_t*u_t)   (fused STT mult+max)
  M finish: PE-transpose amax 128x128 blocks -> DVE reduce_max -> M[k],
    broadcast M to all partitions via ones-row matmul -> M_b.
  pass B (chunk-major over k, per q-tile):
    DVE: ind = (e_t*u_t == M_b)          (fused STT mult+is_equal)
    PE:  colsum[k] += u_t^T @ e_t        (m=1 matmul, psum accum)
         idx[k]    += iota_t^T @ ind     (iota_t[p] = t*128+p, exact int)
  out = colsum @ V  (m=1 matmuls over 16 k-chunks of 128)
"""

import math
import sys

sys.path.insert(0, "/opt/trn_rl_repo")

import numpy as np

import concourse.bass as bass
import concourse.bacc as bacc
import concourse.mybir as mybir
import concourse.tile as tile
from concourse.masks import make_identity

F32 = mybir.dt.float32
I32 = mybir.dt.int32
AF = mybir.ActivationFunctionType
OP = mybir.AluOpType

P = 128  # partitions


def build_nc(bpc=2, Q=2048, K=2048, D=512):
    """Build the per-core Bass program. bpc = batches per core."""
    QT = Q // P  # q tiles
    DC = D // P  # d chunks (contraction)
    KT = K // P  # k tiles of 128 (for V matmul)
    CH = 512  # free-dim chunk for matmuls / psum banks
    KC = K // CH  # k chunks of 512
    HALF = min(1024, K)  # psum s-tile half width
    NH = K // HALF  # halves per q-tile
    scale = 1.0 / math.sqrt(D)

    nc = bacc.Bacc("TRN2", target_bir_lowering=False)
    nc.name = "attn_spmd"

    q_in = nc.dram_tensor("queries", [bpc, Q, D], F32, kind="ExternalInput")
    k_in = nc.dram_tensor("keys", [bpc, K, D], F32, kind="ExternalInput")
    v_in = nc.dram_tensor("values", [bpc, K, D], F32, kind="ExternalInput")
    out_o = nc.dram_tensor("out", [bpc, D], F32, kind="ExternalOutput")
    out_am = nc.dram_tensor("aggr_max", [bpc, K], I32, kind="ExternalOutput")

    # constants baked into the NEFF
    iota_np = np.zeros((P, QT), dtype=np.float32)
    for t in range(QT):
        iota_np[:, t] = t * P + np.arange(P)
    iota_d = nc.inline_tensor(iota_np, name="iota16")
    ones_row_d = nc.inline_tensor(np.ones((1, P), dtype=np.float32), name="ones_row")

    from contextlib import ExitStack

    with tile.TileContext(nc) as tc:
        with ExitStack() as ctx:
            ep = ctx.enter_context
            consts = ep(tc.tile_pool(name="consts", bufs=1))
            kt_pool = ep(tc.tile_pool(name="kt", bufs=DC))
            qt_pool = ep(tc.tile_pool(name="qt", bufs=5))
            e_pool = ep(tc.tile_pool(name="e", bufs=QT))
            amax_pool = ep(tc.tile_pool(name="amax", bufs=2))
            v_pool = ep(tc.tile_pool(name="vst", bufs=2))
            ind_pool = ep(tc.tile_pool(name="ind", bufs=2))
            u_pool = ep(tc.tile_pool(name="u", bufs=2 * QT + 2))
            tiny = ep(tc.tile_pool(name="tiny", bufs=4))
            perbatch = ep(tc.tile_pool(name="perbatch", bufs=1))
            outs_pool = ep(tc.tile_pool(name="outs", bufs=2))
            psA = ep(tc.tile_pool(name="psA", bufs=2, space="PSUM"))
            psT = ep(tc.tile_pool(name="psT", bufs=2, space="PSUM"))
            psAcc = ep(tc.tile_pool(name="psAcc", bufs=2, space="PSUM"))
            dram_pool = ep(tc.tile_pool(name="dram", bufs=2, space="DRAM"))
            iota_sb = consts.tile([P, QT], F32)
            nc.sync.dma_start(out=iota_sb, in_=iota_d[:])
            ones_row = consts.tile([1, P], F32)
            nc.sync.dma_start(out=ones_row, in_=ones_row_d[:])
            ident = consts.tile([P, P], F32)
            make_identity(nc, ident[:])

            for b in range(bpc):
                # --- K^T resident: KT[dc] [128 d, K k] ---
                kts = []
                for dc in range(DC):
                    kt = kt_pool.tile([P, K], F32, tag="kt")
                    nc.sync.dma_start(
                        out=kt,
                        in_=k_in[b, :, dc * P : (dc + 1) * P].rearrange("k d -> d k"),
                    )
                    kts.append(kt)

                es = []
                us = []
                amax = None
                for t in range(QT):
                    qts = []
                    for dc in range(DC):
                        qt_t = qt_pool.tile([P, P], F32, tag="qt")
                        nc.sync.dma_start(
                            out=qt_t,
                            in_=q_in[
                                b, t * P : (t + 1) * P, dc * P : (dc + 1) * P
                            ].rearrange("q d -> d q"),
                        )
                        qts.append(qt_t)

                    e_t = e_pool.tile([P, K], F32, tag="e")
                    rhs_list = []
                    for h in range(NH):
                        ps = psA.tile([P, HALF], F32, tag="s")
                        for dc in range(DC):
                            for c2 in range(HALF // CH):
                                nc.tensor.matmul(
                                    ps[:, c2 * CH : (c2 + 1) * CH],
                                    lhsT=qts[dc],
                                    rhs=kts[dc][
                                        :, h * HALF + c2 * CH : h * HALF + (c2 + 1) * CH
                                    ],
                                    start=(dc == 0),
                                    stop=(dc == DC - 1),
                                )
                        rh = tiny.tile([P, 1], F32, tag="rh")
                        nc.scalar.activation(
                            out=e_t[:, h * HALF : (h + 1) * HALF],
                            in_=ps,
                            func=AF.Exp,
                            scale=scale,
                            accum_out=rh,
                        )
                        rhs_list.append(rh)
                    if NH == 2:
                        r_t = tiny.tile([P, 1], F32, tag="r")
                        nc.vector.tensor_add(r_t, rhs_list[0], rhs_list[1])
                    else:
                        r_t = rhs_list[0]
                    u_t = u_pool.tile([P, 1], F32, tag="u")
                    nc.vector.reciprocal(u_t, r_t)

                    if t == 0:
                        amax = amax_pool.tile([P, K], F32, tag="amax")
                        nc.vector.tensor_scalar_mul(amax, e_t, u_t)
                    else:
                        amax_new = amax_pool.tile([P, K], F32, tag="amax")
                        nc.vector.scalar_tensor_tensor(
                            out=amax_new,
                            in0=e_t,
                            scalar=u_t,
                            in1=amax,
                            op0=OP.mult,
                            op1=OP.max,
                        )
                        amax = amax_new
                    es.append(e_t)
                    us.append(u_t)

                # --- M finish: cross-partition max of amax ---
                m_sb = perbatch.tile([P, KT], F32, tag="msb")
                for j in range(KT):
                    tp = psT.tile([P, P], F32, tag="tp")
                    nc.tensor.transpose(tp, amax[:, j * P : (j + 1) * P], ident)
                    nc.vector.reduce_max(
                        m_sb[:, j : j + 1], tp, axis=mybir.AxisListType.X
                    )
                # M row [1, K]: transpose m_sb -> [QT, 128] -> DRAM -> [1, K]
                tp2 = psT.tile([P, P], F32, tag="tp")
                nc.tensor.transpose(tp2[:KT, :], m_sb, ident)
                m_stage = perbatch.tile([KT, P], F32, tag="mstage")
                nc.vector.tensor_copy(m_stage, tp2[:KT, :])
                m_dram = dram_pool.tile([KT, P], F32, tag="mrow")
                nc.sync.dma_start(out=m_dram, in_=m_stage)
                m_row = perbatch.tile([1, K], F32, tag="mrow_sb")
                nc.sync.dma_start(
                    out=m_row, in_=m_dram.rearrange("t p -> (t p)")[None, :]
                )

                # --- pass B: indicator + colsum + idx accumulation ---
                cs_dram = dram_pool.tile([K], F32, tag="csd")
                for c in range(KC):
                    sl = slice(c * CH, (c + 1) * CH)
                    # broadcast M chunk to all partitions, kept in PSUM
                    mb_ps = psT.tile([P, CH], F32, tag="tp")
                    nc.tensor.matmul(
                        mb_ps,
                        lhsT=ones_row,
                        rhs=m_row[:, sl],
                        start=True,
                        stop=True,
                    )
                    cs_ps = psAcc.tile([1, CH], F32, tag="acc")
                    ix_ps = psAcc.tile([1, CH], F32, tag="acc")
                    for t in range(QT):
                        ind_t = ind_pool.tile([P, CH], F32, tag="ind")
                        nc.vector.scalar_tensor_tensor(
                            out=ind_t,
                            in0=es[t][:, sl],
                            scalar=us[t],
                            in1=mb_ps,
                            op0=OP.mult,
                            op1=OP.is_equal,
                        )
                        nc.tensor.matmul(
                            cs_ps,
                            lhsT=us[t],
                            rhs=es[t][:, sl],
                            start=(t == 0),
                            stop=(t == QT - 1),
                        )
                        nc.tensor.matmul(
                            ix_ps,
                            lhsT=iota_sb[:, t : t + 1],
                            rhs=ind_t,
                            start=(t == 0),
                            stop=(t == QT - 1),
                        )
                    idx_ch = outs_pool.tile([1, CH], I32, tag="idx")
                    cs_ch = outs_pool.tile([1, CH], F32, tag="cs")
                    nc.vector.tensor_copy(idx_ch, ix_ps)
                    nc.vector.tensor_copy(cs_ch, cs_ps)
                    nc.sync.dma_start(out=out_am[b][None, sl], in_=idx_ch)
                    nc.sync.dma_start(out=cs_dram[None, sl], in_=cs_ch)

                # --- out = colsum @ V ---
                colT = perbatch.tile([P, KT], F32, tag="colT")
                nc.sync.dma_start(
                    out=colT, in_=cs_dram.rearrange("(c p) -> p c", p=P)
                )
                out_ps = psT.tile([1, D], F32, tag="tp")
                for kc in range(KT):
                    vt = v_pool.tile([P, D], F32, tag="v")
                    nc.sync.dma_start(out=vt, in_=v_in[b, kc * P : (kc + 1) * P, :])
                    nc.tensor.matmul(
                        out_ps,
                        lhsT=colT[:, kc : kc + 1],
                        rhs=vt,
                        start=(kc == 0),
                        stop=(kc == KT - 1),
                    )
                out_sb = perbatch.tile([1, D], F32, tag="o")
                nc.vector.tensor_copy(out_sb, out_ps)
                nc.sync.dma_start(out=out_o[b][None, :], in_=out_sb)

    nc.finalize()
    return nc


_NC_CACHE = {}


def _get_nc(bpc, Q, K, D):
    key = (bpc, Q, K, D)
    if key not in _NC_CACHE:
        _NC_CACHE[key] = build_nc(bpc, Q, K, D)
    return _NC_CACHE[key]


def run(queries, keys, values, trace=False, **kwargs):
    from concourse.bass_utils import run_bass_kernel_spmd

    B, Q, D = queries.shape
    K = keys.shape[1]
    n_cores = 8
    bpc = B // n_cores
    nc = _get_nc(bpc, Q, K, D)

    queries = np.ascontiguousarray(queries, dtype=np.float32)
    keys = np.ascontiguousarray(keys, dtype=np.float32)
    values = np.ascontiguousarray(values, dtype=np.float32)

    in_maps = [
        {
            "queries": queries[i * bpc : (i + 1) * bpc],
            "keys": keys[i * bpc : (i + 1) * bpc],
            "values": values[i * bpc : (i + 1) * bpc],
        }
        for i in range(n_cores)
    ]
    res = run_bass_kernel_spmd(
        nc, in_maps, core_ids=list(range(n_cores)), trace=trace, **kwargs
    )
    out = np.concatenate([r["out"] for r in res.results], axis=0)
    aggr_max = np.concatenate([r["aggr_max"] for r in res.results], axis=0)
    return (out, aggr_max), res


def kernel(queries, keys, values):
    (out, aggr_max), _ = run(queries, keys, values)
    return out, aggr_max


if __name__ == "__main__":
    nc = build_nc()
    print("built ok")


# revision 25
# speedup vs baseline: 1.9462x; 1.9462x over previous
"""Trainium2 Bass kernel for batched DotProductAttention:
  out[b]      = sum_q softmax(Q K^T / sqrt(d))[q, :] @ V      -> [B, 512] f32
  aggr_max[b] = argmax_q softmax(...)[q, k]                   -> [B, K] int32

B=16 sharded across 8 NeuronCores (2 batches/core), pure data parallel.

Per-core algorithm (natural layout: scores s[q_partition, k_free]):
  Host pre-splits Q,K into fp16 hi+lo (q = qh + ql exactly to ~2^-22), V to
  bf16. Scores are computed as 3 fp16 matmuls (qh kh + qh kl + ql kh) at
  full PE rate (plain fp32 matmul runs at 1/4 rate on TRN2).
  Q^T/K^T tiles are loaded with the XBAR transpose DMA (fp16-only path).

  pass A (per q-tile t, in 512-wide k-chunks):
    PE : s psum = fp16x3 matmuls over 4 d-chunks
    ACT: s_t = Copy(psum * 1/sqrt(d))  -> fp32 SBUF (kept for pass B)
         e   = Exp(psum * 1/sqrt(d))   -> f32r scratch, accum_out -> row sums
    DVE: r_t, u_t = 1/r_t; ACT: nlse_t = Log(u_t)  (same table set as Exp)
         amax = max(s_t + nlse_t, amax)      (fused STT add+max: running max
                                              of log-softmax across tiles)
    PE : colsum psum[1, K] += u_r^T @ e      (f32r matmuls, loose tolerance)
  M finish: PE-transpose amax 128x128 blocks -> DVE reduce_max -> M[k] ->
    DRAM bounce -> M row; gpsimd.partition_broadcast -> M_b (exact copy).
  pass B (per 512-chunk c, per tile t):
    DVE: ind = ((s_t + nlse_t) == M_b) -> bf16 {0,1}
    PE : [t*; p*][2, 512] += tpcols_t^T @ ind   (bf16, exact small ints)
    DVE: idx = 128*t* + p*  -> int32 -> DRAM
  out = colsum @ V in bf16 (colsum bounced via DRAM to [128, KT] layout).
"""

import math
import sys

sys.path.insert(0, "/opt/trn_rl_repo")

import numpy as np

import concourse.bacc as bacc
import concourse.mybir as mybir
import concourse.tile as tile
from concourse.masks import make_identity


def _pin_act_tables():
    """Make Exp/Ln/Copy resolve only to natural_log_exp_and_others so the
    ACT table is loaded once instead of thrashing between sets (set ids are
    list positions, so keep order/length and only edit membership)."""
    import concourse.hw_specs as hw_specs

    orig = hw_specs.get_activation_tables
    pinned = {"Exp", "Ln", "Copy"}

    def patched(module_arch):
        tabs = orig(module_arch)
        AFT = mybir.ActivationFunctionType
        strip = {getattr(AFT, n) for n in pinned}
        out = {}
        for name, fns in tabs.items():
            if name != "natural_log_exp_and_others":
                fns = fns - strip
            out[name] = fns
        return out

    bacc.get_activation_tables = patched


_pin_act_tables()

F32 = mybir.dt.float32
F32R = mybir.dt.float32r
F16 = mybir.dt.float16
BF16 = mybir.dt.bfloat16
I32 = mybir.dt.int32
AF = mybir.ActivationFunctionType
OP = mybir.AluOpType

P = 128


def build_nc(bpc=2, Q=2048, K=2048, D=512):
    QT = Q // P          # q tiles
    DC = D // P          # d chunks (contraction)
    KT = K // P          # k tiles of 128 (V matmul / M blocks)
    CH = 512             # k chunk width (psum bank)
    KC = K // CH         # k chunks
    QG = min(4, QT)      # q tiles per Q^T DMA group
    scale = 1.0 / math.sqrt(D)

    nc = bacc.Bacc("TRN2", target_bir_lowering=False)
    nc.name = "attn_spmd"

    qh_in = nc.dram_tensor("qh", [bpc, Q, D], F16, kind="ExternalInput")
    ql_in = nc.dram_tensor("ql", [bpc, Q, D], F16, kind="ExternalInput")
    kh_in = nc.dram_tensor("kh", [bpc, K, D], F16, kind="ExternalInput")
    kl_in = nc.dram_tensor("kl", [bpc, K, D], F16, kind="ExternalInput")
    v_in = nc.dram_tensor("v16", [bpc, K, D], BF16, kind="ExternalInput")
    out_o = nc.dram_tensor("out", [bpc, D], F32, kind="ExternalOutput")
    out_am = nc.dram_tensor("aggr_max", [bpc, K], I32, kind="ExternalOutput")

    # constants: per-tile [t-const | p-iota] column pairs (exact in bf16)
    tp_np = np.zeros((P, QT), dtype=np.float32)
    for t in range(QT):
        tp_np[:, t] = t * P + np.arange(P)
    tp_d = nc.inline_tensor(tp_np, name="iota_f32")

    from contextlib import ExitStack

    with tile.TileContext(nc) as tc:
        with ExitStack() as ctx:
            ep = ctx.enter_context
            consts = ep(tc.tile_pool(name="consts", bufs=1))
            k16_pool = ep(tc.tile_pool(name="k16", bufs=2 * DC))
            q16_pool = ep(tc.tile_pool(name="q16", bufs=2 * DC + 1))
            s_pool = ep(tc.tile_pool(name="s", bufs=QT))
            amax_pool = ep(tc.tile_pool(name="amax", bufs=2))
            escr_pool = ep(tc.tile_pool(name="escr", bufs=KC))
            ind_pool = ep(tc.tile_pool(name="ind", bufs=2))
            mb_pool = ep(tc.tile_pool(name="mb", bufs=1))
            v_pool = ep(tc.tile_pool(name="vst", bufs=1))
            u_pool = ep(tc.tile_pool(name="u", bufs=2 * QT + 2))
            tiny = ep(tc.tile_pool(name="tiny", bufs=4))
            mrow_pool = ep(tc.tile_pool(name="mrow", bufs=1))
            perbatch = ep(tc.tile_pool(name="perbatch", bufs=1))
            outs_pool = ep(tc.tile_pool(name="outs", bufs=1))
            psA = ep(tc.tile_pool(name="psA", bufs=2, space="PSUM"))
            psCS = ep(tc.tile_pool(name="psCS", bufs=1, space="PSUM"))
            psB = ep(tc.tile_pool(name="psB", bufs=2, space="PSUM"))
            dram_pool = ep(tc.tile_pool(name="dram", bufs=2, space="DRAM"))

            tpcols_f = consts.tile([P, QT], F32)
            nc.sync.dma_start(out=tpcols_f, in_=tp_d[:])
            tpcols = consts.tile([P, QT], F16)
            nc.vector.tensor_copy(tpcols, tpcols_f)
            ident = consts.tile([P, P], F32)
            make_identity(nc, ident[:])

            for b in range(bpc):
                # K^T resident, fp16 hi/lo per d-chunk: [128 d, K]
                kts = []  # kts[dc] = (kh_tile, kl_tile)
                for dc in range(DC):
                    pair = []
                    for src in (kh_in, kl_in):
                        kt = k16_pool.tile([P, K], F16, tag="k16")
                        nc.sync.dma_start_transpose(
                            kt, src[b, :, dc * P : (dc + 1) * P]
                        )
                        pair.append(kt)
                    kts.append(pair)

                ss = []
                nlses = []
                amax = None
                cs_ps = psCS.tile([1, K], F32, tag="cs")
                qg_tiles = None
                for t in range(QT):
                    if t % QG == 0:
                        g0 = t * P
                        qg_tiles = []
                        for dc in range(DC):
                            pair = []
                            for src in (qh_in, ql_in):
                                qt16 = q16_pool.tile([P, QG * P], F16, tag="q16")
                                nc.sync.dma_start_transpose(
                                    qt16,
                                    src[b, g0 : g0 + QG * P, dc * P : (dc + 1) * P],
                                )
                                pair.append(qt16)
                            qg_tiles.append(pair)
                    qsl = slice((t % QG) * P, (t % QG) * P + P)

                    s_t = s_pool.tile([P, K], F32, tag="s")
                    rh = tiny.tile([P, KC], F32, tag="rh")
                    escrs = []
                    for c in range(KC):
                        ksl = slice(c * CH, (c + 1) * CH)
                        ps = psA.tile([P, CH], F32, tag="sq")
                        for idc, dc in enumerate(range(DC)):
                            qh_t = qg_tiles[dc][0][:, qsl]
                            ql_t = qg_tiles[dc][1][:, qsl]
                            kh_t = kts[dc][0][:, ksl]
                            kl_t = kts[dc][1][:, ksl]
                            for ip, (lt, rt) in enumerate(
                                ((qh_t, kh_t), (qh_t, kl_t), (ql_t, kh_t))
                            ):
                                nc.tensor.matmul(
                                    ps,
                                    lhsT=lt,
                                    rhs=rt,
                                    start=(idc == 0 and ip == 0),
                                    stop=(idc == DC - 1 and ip == 2),
                                )
                        nc.scalar.activation(
                            out=s_t[:, ksl], in_=ps, func=AF.Copy, scale=scale
                        )
                        e_c = escr_pool.tile([P, CH], F32R, tag="e")
                        nc.scalar.activation(
                            out=e_c,
                            in_=ps,
                            func=AF.Exp,
                            scale=scale,
                            accum_out=rh[:, c : c + 1],
                        )
                        escrs.append(e_c)

                    if KC > 1:
                        r_t = tiny.tile([P, 1], F32, tag="r")
                        nc.vector.reduce_sum(r_t, rh, axis=mybir.AxisListType.X)
                    else:
                        r_t = rh
                    u_t = tiny.tile([P, 1], F32, tag="ut")
                    nc.vector.reciprocal(u_t, r_t)
                    u_r = u_pool.tile([P, 1], F32R, tag="ur")
                    nc.vector.tensor_copy(u_r, u_t)
                    nlse_t = u_pool.tile([P, 1], F32, tag="nlse")
                    nc.scalar.activation(out=nlse_t, in_=u_t, func=AF.Ln)

                    for c in range(KC):
                        nc.tensor.matmul(
                            cs_ps[:, c * CH : (c + 1) * CH],
                            lhsT=u_r,
                            rhs=escrs[c],
                            start=(t == 0),
                            stop=(t == QT - 1),
                        )

                    if t == 0:
                        amax = amax_pool.tile([P, K], F32, tag="amax")
                        nc.vector.tensor_scalar(
                            amax, s_t, nlse_t, scalar2=None, op0=OP.add
                        )
                    else:
                        amax_new = amax_pool.tile([P, K], F32, tag="amax")
                        nc.vector.scalar_tensor_tensor(
                            out=amax_new,
                            in0=s_t,
                            scalar=nlse_t,
                            in1=amax,
                            op0=OP.add,
                            op1=OP.max,
                        )
                        amax = amax_new
                    ss.append(s_t)
                    nlses.append(nlse_t)

                # ---- M finish: cross-partition max of amax -> M row in DRAM
                m_sb = perbatch.tile([P, KT], F32, tag="msb")
                for j in range(KT):
                    tp = psB.tile([P, CH], F32, tag="misc")
                    nc.tensor.transpose(
                        tp[:, :P], amax[:, j * P : (j + 1) * P], ident
                    )
                    nc.vector.reduce_max(
                        m_sb[:, j : j + 1], tp[:, :P], axis=mybir.AxisListType.X
                    )
                tp2 = psB.tile([P, CH], F32, tag="misc")
                nc.tensor.transpose(tp2[:KT, :P], m_sb, ident)
                m_stage = perbatch.tile([KT, P], F32, tag="mstage")
                nc.vector.tensor_copy(m_stage, tp2[:KT, :P])
                m_dram = dram_pool.tile([KT, P], F32, tag="mrow")
                nc.sync.dma_start(out=m_dram, in_=m_stage)

                # colsum out of PSUM -> DRAM (frees psCS for next batch)
                cs_dram = dram_pool.tile([K], F32, tag="csd")
                for c in range(KC):
                    sl = slice(c * CH, (c + 1) * CH)
                    cs_st = outs_pool.tile([1, CH], F32, tag="stg")
                    nc.vector.tensor_copy(cs_st, cs_ps[:, sl])
                    nc.sync.dma_start(out=cs_dram[None, sl], in_=cs_st)

                # ---- pass B: indicator + argmax index extraction
                mrow_flat = m_dram.rearrange("t p -> (t p)")
                for c in range(KC):
                    sl = slice(c * CH, (c + 1) * CH)
                    mrow_c = mrow_pool.tile([1, CH], F32, tag="mrow")
                    nc.sync.dma_start(out=mrow_c, in_=mrow_flat[None, sl])
                    m_b = mb_pool.tile([P, CH], F32, tag="mb")
                    nc.gpsimd.partition_broadcast(m_b, mrow_c, channels=P)
                    tp_ps = psB.tile([1, CH], F32, tag="misc")
                    for t in range(QT):
                        ind_t = ind_pool.tile([P, CH], F16, tag="ind")
                        nc.vector.scalar_tensor_tensor(
                            out=ind_t,
                            in0=ss[t][:, sl],
                            scalar=nlses[t],
                            in1=m_b,
                            op0=OP.add,
                            op1=OP.is_equal,
                        )
                        nc.tensor.matmul(
                            tp_ps,
                            lhsT=tpcols[:, t : t + 1],
                            rhs=ind_t,
                            start=(t == 0),
                            stop=(t == QT - 1),
                        )
                    idx_c = outs_pool.tile([1, CH], I32, tag="stg")
                    nc.vector.tensor_copy(idx_c, tp_ps)
                    nc.sync.dma_start(out=out_am[b][None, sl], in_=idx_c)

                # ---- out = colsum @ V (bf16)
                colT_f = perbatch.tile([P, KT], F32, tag="colTf")
                nc.sync.dma_start(
                    out=colT_f, in_=cs_dram.rearrange("(c p) -> p c", p=P)
                )
                colT = perbatch.tile([P, KT], BF16, tag="colT")
                nc.vector.tensor_copy(colT, colT_f)
                out_ps = psB.tile([1, D], F32, tag="misc")
                for kc in range(KT):
                    vt = v_pool.tile([P, D], BF16, tag="v")
                    nc.sync.dma_start(out=vt, in_=v_in[b, kc * P : (kc + 1) * P, :])
                    nc.tensor.matmul(
                        out_ps,
                        lhsT=colT[:, kc : kc + 1],
                        rhs=vt,
                        start=(kc == 0),
                        stop=(kc == KT - 1),
                    )
                out_sb = perbatch.tile([1, D], F32, tag="o")
                nc.vector.tensor_copy(out_sb, out_ps)
                nc.sync.dma_start(out=out_o[b][None, :], in_=out_sb)

    nc.finalize()
    return nc


_NC_CACHE = {}


def _get_nc(bpc, Q, K, D):
    key = (bpc, Q, K, D)
    if key not in _NC_CACHE:
        _NC_CACHE[key] = build_nc(bpc, Q, K, D)
    return _NC_CACHE[key]


def prep_inputs(queries, keys, values):
    """Host-side split: q = qh + ql exactly to fp16x2 precision; v to bf16."""
    import ml_dtypes

    q32 = np.ascontiguousarray(queries, dtype=np.float32)
    k32 = np.ascontiguousarray(keys, dtype=np.float32)
    qh = q32.astype(np.float16)
    ql = (q32 - qh.astype(np.float32)).astype(np.float16)
    kh = k32.astype(np.float16)
    kl = (k32 - kh.astype(np.float32)).astype(np.float16)
    v16 = np.ascontiguousarray(values, dtype=np.float32).astype(ml_dtypes.bfloat16)
    return qh, ql, kh, kl, v16


def run(queries, keys, values, trace=False, **kwargs):
    from concourse.bass_utils import run_bass_kernel_spmd

    B, Q, D = queries.shape
    K = keys.shape[1]
    n_cores = 8
    bpc = B // n_cores
    nc = _get_nc(bpc, Q, K, D)
    qh, ql, kh, kl, v16 = prep_inputs(queries, keys, values)

    in_maps = [
        {
            "qh": qh[i * bpc : (i + 1) * bpc],
            "ql": ql[i * bpc : (i + 1) * bpc],
            "kh": kh[i * bpc : (i + 1) * bpc],
            "kl": kl[i * bpc : (i + 1) * bpc],
            "v16": v16[i * bpc : (i + 1) * bpc],
        }
        for i in range(n_cores)
    ]
    res = run_bass_kernel_spmd(
        nc, in_maps, core_ids=list(range(n_cores)), trace=trace, **kwargs
    )
    out = np.concatenate([r["out"] for r in res.results], axis=0)
    aggr_max = np.concatenate([r["aggr_max"] for r in res.results], axis=0)
    return (out, aggr_max), res


def kernel(queries, keys, values):
    (out, aggr_max), _ = run(queries, keys, values)
    return out, aggr_max


if __name__ == "__main__":
    nc = build_nc()
    print("built ok")
